# revision 34
# baseline (speedup 1.0000x reference)
"""Trainium2 Bass kernel for nn_MembraneLayer: h = x @ w followed by a
double first-order recurrence over time, producing (syn_rec, mem_rec).

Sharding: data-parallel over batch. 8 cores x 64 batches each.

Per-core device layout (all hardcoded):
  xth   [700, 6400]  fp16  x transposed to [C, b*T+t] (host-prepped)
  w16   [700, 512]   fp16
  coefs [128, 8]     f32   cols 0-3: alpha per d_tile; cols 4-7: beta
  syn   [512, 6400]  fp16  out: U[t] = a*U[t-1] + h[t], stored shifted +1
  mem   [512, 6400]  fp16  out: W[t] = b*W[t-1] + U[t], stored shifted +2
                           (host scales by (1-beta) and zeroes t<shift)

Work is chunked over batches per CHUNKS below (a group = one PSUM bank,
500 time-slots = 5 batches, one 400-slot remainder).  Per chunk and
d_tile, six fp16 matmul k-passes accumulate h into per-bank PSUM groups
(a matmul output AP may not leave its bank); one DVE tensor_tensor_scan
per bank computes U from PSUM, one scan per chunk computes W from U.
Coefficient tiles (alpha/beta broadcast along time with 0 at batch-start
columns to reset the recurrence across the batch concatenation) are
built on the otherwise-idle scalar engine; memsets run on the idle
gpsimd.  Scan state is fp32 internally regardless of the fp16 output
dtype, so the recurrence does not accumulate fp16 error; coefficients
stay f32 (fp16 coef rounding would be amplified ~(1-a)^-1 through the
recurrence — measured 4e-2 relmax on mem, over the 2e-2 gate).

The DVE scans are the bottleneck (~117us busy of ~140us): the stock
TensorTensorScanArith runs at ~2 DVE cycles per column (latency-bound on
the mult+add feedback chain; 16-bit operands do NOT engage the 2x perf
mode — measured).  Scans are DVE-only (the ISA rejects them on Pool),
custom-DVE specs cannot nest scan() or reset per page, and there is no
FMA ALU stage, so ~2cyc/col x 51200 cols is a hard floor here.
"""

import os

import numpy as np

import concourse.bass as bass
import concourse.tile as tile
from concourse import bacc, mybir
from concourse import bass_utils

B, T, C, D = 512, 100, 700, 512
NCORES = 8
BC = B // NCORES  # 64
# chunks of PSUM-bank-sized groups, as per-group column counts; a group is
# 5 batches (500 slots) except one 400-slot remainder (64 batches total).
# Fewer, wider groups amortize the ~140ns fixed cost per DVE scan; the
# small last chunk shortens the pipeline tail.
# NOTE: small leading chunks were tried twice and consistently REGRESSED
# (~5-7us): the DVE drains the small chunk before the next chunk's matmuls
# (running at the HAM-throttled early clock) can refill the pipeline.
# Big chunks first, small chunk last (short tail) wins.
CHUNKS = [[500, 500], [500, 500, 500], [500, 500, 500, 500], [500, 500, 500], [400]]
CW = 2000  # max chunk width (tile allocation size)
KT = [(k * 128, min(128, C - k * 128)) for k in range(6)]
F32 = mybir.dt.float32
FP16 = mybir.dt.float16

LAST_RESULT = None
_cache = {}


def _build():
    key = "nc"
    if key in _cache:
        return _cache[key]
    nc = bacc.Bacc("TRN2", target_bir_lowering=False, debug=False)

    xt_d = nc.dram_tensor("xth", [C, BC * T], FP16, kind="ExternalInput").ap()
    w_d = nc.dram_tensor("w16", [C, D], FP16, kind="ExternalInput").ap()
    cf_d = nc.dram_tensor("coefs", [128, 8], F32, kind="ExternalInput").ap()
    syn_d = nc.dram_tensor("syn", [D, BC * T], FP16, kind="ExternalOutput").ap()
    mem_d = nc.dram_tensor("mem", [D, BC * T], FP16, kind="ExternalOutput").ap()

    with tile.TileContext(nc) as tc:
        from contextlib import ExitStack

        with ExitStack() as ctx:
            cpool = ctx.enter_context(tc.tile_pool(name="consts", bufs=1))
            # warmup scratch: memset first on the DVE (boots earliest) so the
            # PE warmup below is runnable the moment the tensor engine is up
            warm_sb = cpool.tile([128, 512], FP16, name="warm", tag="warm")
            nc.vector.memset(warm_sb[:], 0.0)
            cf_t = cpool.tile([128, 8], F32, name="cf", tag="cf")
            # tiny load on the gpsimd (SWDGE) queue so the sync queue leads
            # with the first x tiles
            nc.gpsimd.dma_start(cf_t[:], cf_d)
            w_tiles = [
                cpool.tile([128, D], FP16, name=f"w{k}", tag=f"w{k}")
                for k in range(len(KT))
            ]

            # coef tiles: alpha/beta broadcast along CW cols, 0 at each
            # batch-start column; built on the scalar engine from cf
            ones = cpool.tile([128, CW], F32, name="ones", tag="ones")
            nc.gpsimd.memset(ones[:], 1.0)
            ac_t, bc_t = [], []
            for i in range(4):
                a = cpool.tile([128, CW], F32, name=f"ac{i}", tag=f"ac{i}")
                nc.scalar.mul(a[:], ones[:], cf_t[:, i : i + 1])
                nc.gpsimd.memset(
                    a[:].rearrange("p (b t) -> p b t", t=100)[:, :, 0:1], 0.0
                )
                ac_t.append(a)
                b = cpool.tile([128, CW], F32, name=f"bc{i}", tag=f"bc{i}")
                nc.scalar.mul(b[:], ones[:], cf_t[:, 4 + i : 5 + i])
                nc.gpsimd.memset(
                    b[:].rearrange("p (b t) -> p b t", t=100)[:, :, 0:1], 0.0
                )
                bc_t.append(b)

            xp = ctx.enter_context(tc.tile_pool(name="xp", bufs=2))
            pp = ctx.enter_context(tc.tile_pool(name="pp", bufs=2, space="PSUM"))
            up = ctx.enter_context(tc.tile_pool(name="up", bufs=3))
            vp = ctx.enter_context(tc.tile_pool(name="vp", bufs=3))

            # PE warmup: dummy matmuls on scratch data run during the initial
            # DMA wait, so HAM un-throttles (K=8/8) before the first real MM
            warm_ps = pp.tile([128, 2048], F32, tag="ps", name="warm_ps")
            for _ in range(16):
                nc.tensor.matmul(
                    warm_ps[:, 0:512], warm_sb[:, 0:128], warm_sb[:],
                    start=True, stop=True,
                )

            c0 = 0
            for q, widths in enumerate(CHUNKS):
                cols = sum(widths)
                offs = [sum(widths[:g]) for g in range(len(widths))]
                xts = []
                for k, (r0, rk) in enumerate(KT):
                    # interleave x/w loads so matmul k can start as soon as
                    # its own pair has landed; x first (the larger transfer)
                    t_ = xp.tile([128, CW], FP16, tag=f"x{k}", name=f"x{k}_{q}")
                    nc.sync.dma_start(
                        t_[:rk, :cols], xt_d[r0 : r0 + rk, c0 : c0 + cols]
                    )
                    xts.append(t_)
                    if q == 0:
                        nc.sync.dma_start(w_tiles[k][:rk, :], w_d[r0 : r0 + rk, :])

                for di in range(4):
                    dsl = slice(di * 128, (di + 1) * 128)
                    ps = pp.tile([128, 2048], F32, tag="ps", name=f"ps_{q}_{di}")
                    for k, (r0, rk) in enumerate(KT):
                        lhsT = w_tiles[k][:rk, dsl]
                        for g, wg in enumerate(widths):
                            # one group per 512-col PSUM bank: a matmul
                            # output AP may not leave its bank (3D
                            # bank-strided out is invalid ISA; contiguous
                            # cross-bank accumulation corrupts results)
                            nc.tensor.matmul(
                                ps[:, g * 512 : g * 512 + wg],
                                lhsT,
                                xts[k][:rk, offs[g] : offs[g] + wg],
                                start=(k == 0),
                                stop=(k == len(KT) - 1),
                            )

                    # syn: U[t] = alpha*U[t-1] + h[t], one scan per bank
                    # (scan APs must be 2D, so the banked groups cannot be
                    # merged); stored shifted by one (syn_rec[t+1] = U[t])
                    u = up.tile([128, CW], FP16, tag="u", name=f"u_{q}_{di}")
                    for g, wg in enumerate(widths):
                        nc.vector.tensor_tensor_scan(
                            u[:, offs[g] : offs[g] + wg],
                            ac_t[di][:, :wg],
                            ps[:, g * 512 : g * 512 + wg],
                            0.0,
                            mybir.AluOpType.mult,
                            mybir.AluOpType.add,
                        )
                    nc.scalar.dma_start(
                        syn_d[dsl, c0 + 1 : c0 + cols], u[:, : cols - 1]
                    )
                    # mem/(1-beta): W[t] = beta*W[t-1] + U[t]; stored shifted
                    # by two (mem_rec[t+2] = (1-beta)*W[t], scaled on host)
                    v = vp.tile([128, CW], FP16, tag="v", name=f"v_{q}_{di}")
                    nc.vector.tensor_tensor_scan(
                        v[:, :cols],
                        bc_t[di][:, :cols],
                        u[:, :cols],
                        0.0,
                        mybir.AluOpType.mult,
                        mybir.AluOpType.add,
                    )
                    nc.scalar.dma_start(
                        mem_d[dsl, c0 + 2 : c0 + cols], v[:, : cols - 2]
                    )
                c0 += cols

    nc.compile()
    _cache[key] = nc
    return nc


def kernel(inputs, w, alpha, beta):
    global LAST_RESULT
    inputs = np.asarray(inputs, dtype=np.float32)
    w = np.asarray(w, dtype=np.float32)
    alpha = np.asarray(alpha, dtype=np.float32).reshape(-1)
    beta = np.asarray(beta, dtype=np.float32).reshape(-1)

    nc = _build()

    coefs = np.concatenate(
        [alpha.reshape(4, 128).T, beta.reshape(4, 128).T], axis=1
    ).astype(np.float32)  # [128, 8]
    omb_col = (1.0 - beta).reshape(D, 1)  # host-side scale for mem
    w16 = w.astype(np.float16)

    in_maps = []
    for c in range(NCORES):
        xc = inputs[c * BC : (c + 1) * BC]  # [64, 100, 700]
        xth = xc.reshape(BC * T, C).T.astype(np.float16)  # [700, 6400]
        in_maps.append({"xth": xth, "w16": w16, "coefs": coefs})

    run_kwargs = {}
    if os.environ.get("MEMBRANE_TRACE_DIR"):
        run_kwargs["tmpdir"] = os.environ["MEMBRANE_TRACE_DIR"]
    res = bass_utils.run_bass_kernel_spmd(
        nc, in_maps, core_ids=list(range(NCORES)), **run_kwargs
    )
    LAST_RESULT = res

    syn_full = np.empty((B, T, D), dtype=np.float32)
    mem_full = np.empty((B, T, D), dtype=np.float32)
    for c in range(NCORES):
        r = res.results[c]
        syn_full[c * BC : (c + 1) * BC] = (
            r["syn"].astype(np.float32).reshape(D, BC, T).transpose(1, 2, 0)
        )
        mem_full[c * BC : (c + 1) * BC] = (
            (r["mem"].astype(np.float32) * omb_col)
            .reshape(D, BC, T)
            .transpose(1, 2, 0)
        )
    syn_full[:, 0, :] = 0.0
    mem_full[:, 0:2, :] = 0.0
    return (syn_full, mem_full)


# revision 35
# speedup vs baseline: 1.0042x; 1.0042x over previous
"""Trainium2 Bass kernel for nn_MembraneLayer: h = x @ w followed by a
double first-order recurrence over time, producing (syn_rec, mem_rec).

Sharding: data-parallel over batch. 8 cores x 64 batches each.

Per-core device layout (all hardcoded):
  xth   [700, 6400]  fp16  x transposed to [C, b*T+t] (host-prepped)
  w16   [700, 512]   fp16
  coefs [128, 8]     f32   cols 0-3: alpha per d_tile; cols 4-7: beta
  syn   [512, 6400]  fp16  out: U[t] = a*U[t-1] + h[t], stored shifted +1
  mem   [512, 6400]  fp16  out: W[t] = b*W[t-1] + U[t], stored shifted +2
                           (host scales by (1-beta) and zeroes t<shift)

Work is chunked over batches per CHUNKS below (a group = one PSUM bank,
500 time-slots = 5 batches, one 400-slot remainder).  Per chunk and
d_tile, six fp16 matmul k-passes accumulate h into per-bank PSUM groups
(a matmul output AP may not leave its bank); one DVE tensor_tensor_scan
per bank computes U from PSUM, one scan per chunk computes W from U.
Coefficient tiles (alpha/beta broadcast along time with 0 at batch-start
columns to reset the recurrence across the batch concatenation) are
built on the otherwise-idle scalar engine; memsets run on the idle
gpsimd.  Scan state is fp32 internally regardless of the fp16 output
dtype, so the recurrence does not accumulate fp16 error; coefficients
stay f32 (fp16 coef rounding would be amplified ~(1-a)^-1 through the
recurrence — measured 4e-2 relmax on mem, over the 2e-2 gate).

The DVE scans are the bottleneck (~117us busy of ~140us): the stock
TensorTensorScanArith runs at ~2 DVE cycles per column (latency-bound on
the mult+add feedback chain; 16-bit operands do NOT engage the 2x perf
mode — measured).  Scans are DVE-only (the ISA rejects them on Pool),
custom-DVE specs cannot nest scan() or reset per page, and there is no
FMA ALU stage, so ~2cyc/col x 51200 cols is a hard floor here.
"""

import os

import numpy as np

import concourse.bass as bass
import concourse.tile as tile
from concourse import bacc, mybir
from concourse import bass_utils

B, T, C, D = 512, 100, 700, 512
NCORES = 8
BC = B // NCORES  # 64
# chunks of PSUM-bank-sized groups, as per-group column counts; a group is
# 5 batches (500 slots) except one 400-slot remainder (64 batches total).
# Fewer, wider groups amortize the ~140ns fixed cost per DVE scan; the
# small last chunk shortens the pipeline tail.
# NOTE: small leading chunks were tried twice and consistently REGRESSED
# (~5-7us): the DVE drains the small chunk before the next chunk's matmuls
# (running at the HAM-throttled early clock) can refill the pipeline.
# Big chunks first, small chunk last (short tail) wins.
CHUNKS = [[500, 500], [500, 500, 500], [500, 500, 500, 500], [500, 500, 500], [400]]
CW = 2000  # max chunk width (tile allocation size)
KT = [(k * 128, min(128, C - k * 128)) for k in range(6)]
F32 = mybir.dt.float32
FP16 = mybir.dt.float16

LAST_RESULT = None
_cache = {}


def _build():
    key = "nc"
    if key in _cache:
        return _cache[key]
    nc = bacc.Bacc("TRN2", target_bir_lowering=False, debug=False)

    xt_d = nc.dram_tensor("xth", [C, BC * T], FP16, kind="ExternalInput").ap()
    w_d = nc.dram_tensor("w16", [C, D], FP16, kind="ExternalInput").ap()
    cf_d = nc.dram_tensor("coefs", [128, 8], F32, kind="ExternalInput").ap()
    syn_d = nc.dram_tensor("syn", [D, BC * T], FP16, kind="ExternalOutput").ap()
    mem_d = nc.dram_tensor("mem", [D, BC * T], FP16, kind="ExternalOutput").ap()

    with tile.TileContext(nc) as tc:
        from contextlib import ExitStack

        with ExitStack() as ctx:
            cpool = ctx.enter_context(tc.tile_pool(name="consts", bufs=1))
            # warmup scratch: memset first on the DVE (boots earliest) so the
            # PE warmup below is runnable the moment the tensor engine is up
            warm_sb = cpool.tile([128, 512], FP16, name="warm", tag="warm")
            nc.vector.memset(warm_sb[:], 0.0)
            cf_t = cpool.tile([128, 8], F32, name="cf", tag="cf")
            # tiny load on the gpsimd (SWDGE) queue so the sync queue leads
            # with the first x tiles
            nc.gpsimd.dma_start(cf_t[:], cf_d)
            w_tiles = [
                cpool.tile([128, D], FP16, name=f"w{k}", tag=f"w{k}")
                for k in range(len(KT))
            ]

            # coef tiles: alpha/beta broadcast along CW cols, 0 at each
            # batch-start column; built on the scalar engine from cf
            ones = cpool.tile([128, CW], F32, name="ones", tag="ones")
            nc.gpsimd.memset(ones[:], 1.0)
            ac_t, bc_t = [], []
            for i in range(4):
                a = cpool.tile([128, CW], F32, name=f"ac{i}", tag=f"ac{i}")
                nc.scalar.mul(a[:], ones[:], cf_t[:, i : i + 1])
                nc.gpsimd.memset(
                    a[:].rearrange("p (b t) -> p b t", t=100)[:, :, 0:1], 0.0
                )
                ac_t.append(a)
                b = cpool.tile([128, CW], F32, name=f"bc{i}", tag=f"bc{i}")
                nc.scalar.mul(b[:], ones[:], cf_t[:, 4 + i : 5 + i])
                nc.gpsimd.memset(
                    b[:].rearrange("p (b t) -> p b t", t=100)[:, :, 0:1], 0.0
                )
                bc_t.append(b)

            xp = ctx.enter_context(tc.tile_pool(name="xp", bufs=2))
            pp = ctx.enter_context(tc.tile_pool(name="pp", bufs=2, space="PSUM"))
            up = ctx.enter_context(tc.tile_pool(name="up", bufs=3))
            vp = ctx.enter_context(tc.tile_pool(name="vp", bufs=3))

            # PE warmup: dummy matmuls on scratch data run during the initial
            # DMA wait, so HAM un-throttles (K=8/8) before the first real MM
            warm_ps = pp.tile([128, 2048], F32, tag="ps", name="warm_ps")
            for _ in range(16):
                nc.tensor.matmul(
                    warm_ps[:, 0:512], warm_sb[:, 0:128], warm_sb[:],
                    start=True, stop=True,
                )

            c0 = 0
            for q, widths in enumerate(CHUNKS):
                cols = sum(widths)
                offs = [sum(widths[:g]) for g in range(len(widths))]
                xts = []
                for k, (r0, rk) in enumerate(KT):
                    # interleave x/w loads so matmul k can start as soon as
                    # its own pair has landed; x first (the larger transfer)
                    t_ = xp.tile([128, CW], FP16, tag=f"x{k}", name=f"x{k}_{q}")
                    nc.sync.dma_start(
                        t_[:rk, :cols], xt_d[r0 : r0 + rk, c0 : c0 + cols]
                    )
                    xts.append(t_)
                    if q == 0:
                        nc.sync.dma_start(w_tiles[k][:rk, :], w_d[r0 : r0 + rk, :])

                for di in range(4):
                    dsl = slice(di * 128, (di + 1) * 128)
                    ps = pp.tile([128, 2048], F32, tag="ps", name=f"ps_{q}_{di}")
                    for k, (r0, rk) in enumerate(KT):
                        lhsT = w_tiles[k][:rk, dsl]
                        for g, wg in enumerate(widths):
                            # one group per 512-col PSUM bank: a matmul
                            # output AP may not leave its bank (3D
                            # bank-strided out is invalid ISA; contiguous
                            # cross-bank accumulation corrupts results)
                            nc.tensor.matmul(
                                ps[:, g * 512 : g * 512 + wg],
                                lhsT,
                                xts[k][:rk, offs[g] : offs[g] + wg],
                                start=(k == 0),
                                stop=(k == len(KT) - 1),
                            )

                    # syn: U[t] = alpha*U[t-1] + h[t], one scan per bank
                    # (scan APs must be 2D, so the banked groups cannot be
                    # merged); stored shifted by one (syn_rec[t+1] = U[t])
                    u = up.tile([128, CW], FP16, tag="u", name=f"u_{q}_{di}")
                    for g, wg in enumerate(widths):
                        nc.vector.tensor_tensor_scan(
                            u[:, offs[g] : offs[g] + wg],
                            ac_t[di][:, :wg],
                            ps[:, g * 512 : g * 512 + wg],
                            0.0,
                            mybir.AluOpType.mult,
                            mybir.AluOpType.add,
                        )
                    # syn stores ride the sync queue (idle once x loading
                    # tails off), mem stores the scalar queue — two queues
                    # drain the 13MB of stores in parallel
                    nc.sync.dma_start(
                        syn_d[dsl, c0 + 1 : c0 + cols], u[:, : cols - 1]
                    )
                    # mem/(1-beta): W[t] = beta*W[t-1] + U[t]; stored shifted
                    # by two (mem_rec[t+2] = (1-beta)*W[t], scaled on host)
                    v = vp.tile([128, CW], FP16, tag="v", name=f"v_{q}_{di}")
                    nc.vector.tensor_tensor_scan(
                        v[:, :cols],
                        bc_t[di][:, :cols],
                        u[:, :cols],
                        0.0,
                        mybir.AluOpType.mult,
                        mybir.AluOpType.add,
                    )
                    nc.scalar.dma_start(
                        mem_d[dsl, c0 + 2 : c0 + cols], v[:, : cols - 2]
                    )
                c0 += cols

    nc.compile()
    _cache[key] = nc
    return nc


def kernel(inputs, w, alpha, beta):
    global LAST_RESULT
    inputs = np.asarray(inputs, dtype=np.float32)
    w = np.asarray(w, dtype=np.float32)
    alpha = np.asarray(alpha, dtype=np.float32).reshape(-1)
    beta = np.asarray(beta, dtype=np.float32).reshape(-1)

    nc = _build()

    coefs = np.concatenate(
        [alpha.reshape(4, 128).T, beta.reshape(4, 128).T], axis=1
    ).astype(np.float32)  # [128, 8]
    omb_col = (1.0 - beta).reshape(D, 1)  # host-side scale for mem
    w16 = w.astype(np.float16)

    in_maps = []
    for c in range(NCORES):
        xc = inputs[c * BC : (c + 1) * BC]  # [64, 100, 700]
        xth = xc.reshape(BC * T, C).T.astype(np.float16)  # [700, 6400]
        in_maps.append({"xth": xth, "w16": w16, "coefs": coefs})

    run_kwargs = {}
    if os.environ.get("MEMBRANE_TRACE_DIR"):
        run_kwargs["tmpdir"] = os.environ["MEMBRANE_TRACE_DIR"]
    res = bass_utils.run_bass_kernel_spmd(
        nc, in_maps, core_ids=list(range(NCORES)), **run_kwargs
    )
    LAST_RESULT = res

    syn_full = np.empty((B, T, D), dtype=np.float32)
    mem_full = np.empty((B, T, D), dtype=np.float32)
    for c in range(NCORES):
        r = res.results[c]
        syn_full[c * BC : (c + 1) * BC] = (
            r["syn"].astype(np.float32).reshape(D, BC, T).transpose(1, 2, 0)
        )
        mem_full[c * BC : (c + 1) * BC] = (
            (r["mem"].astype(np.float32) * omb_col)
            .reshape(D, BC, T)
            .transpose(1, 2, 0)
        )
    syn_full[:, 0, :] = 0.0
    mem_full[:, 0:2, :] = 0.0
    return (syn_full, mem_full)
